# revision 15
# baseline (speedup 1.0000x reference)
"""Trainium2 Bass kernel for nn_BNN1D_14448269984213.

Math note (exact algebraic simplification of the reference network):
  bsign(x) = +1 for x >= 0, and every bin_act() in the reference is applied
  to a post-ReLU / post-maxpool / post-mean tensor, which is elementwise >= 0.
  Therefore each binarized activation is the constant tensor  s * ones, and
  the network output is batch-independent:

      a4  = sa3 * ones[B, 128]                      (input of bin_fc)
      h4  = a4 @ (bsign(wf) * max|wf|).T + bf       = sa3*max|wf|*rowsum(bsign(wf)) + bf
      r4  = relu(batchnorm(h4; g4, be4, m4, v4))
      out = r4 @ wl.T + bl                          (same 10-vector for every row)

  This identity holds for arbitrary values of every input tensor (verified
  against a direct-convolution implementation of the full reference), so the
  kernel computes the exact reference output for any inputs with these shapes.

Sharding: pure data parallel over the batch. Each of the 8 cores computes its
own 64-row output shard [10, 64] on device from the (replicated, tiny) weights;
the host transposes/concatenates the shards into the full [512, 10] output.

Implementation: raw Bass (explicit engine blocks + semaphores). The Tile
framework's auto-generated epilogue (multi-wait Drain / EVENT_SEMAPHORE_
RANGE_CLEAR) does not compile with the walrus build in this environment, and
its embedded multi-wait sync attachments exceed the one-sync-wait capacity of
TRN2 instruction encodings. Raw Bass uses standalone sequencer waits instead.
"""

from contextlib import ExitStack

import numpy as np

import concourse.bass as bass
import concourse.mybir as mybir
from concourse.bass_utils import run_bass_kernel_spmd

F32 = mybir.dt.float32
ALU = mybir.AluOpType
AX = mybir.AxisListType
ACT = mybir.ActivationFunctionType

EPS = 1e-5
N_CORES = 8
B = 512
B_SHARD = B // N_CORES  # 64
CF = 128  # bin_fc in features
CO = 64   # bin_fc out features
NCLS = 10


def build_kernel() -> bass.Bass:
    nc = bass.Bass()

    wf_d = nc.declare_dram_parameter("wf", [CO, CF], F32, isOutput=False)
    bf_d = nc.declare_dram_parameter("bf", [CO], F32, isOutput=False)
    g4_d = nc.declare_dram_parameter("g4", [CO], F32, isOutput=False)
    be4_d = nc.declare_dram_parameter("be4", [CO], F32, isOutput=False)
    m4_d = nc.declare_dram_parameter("m4", [CO], F32, isOutput=False)
    v4_d = nc.declare_dram_parameter("v4", [CO], F32, isOutput=False)
    wl_d = nc.declare_dram_parameter("wl", [NCLS, CO], F32, isOutput=False)
    bl_d = nc.declare_dram_parameter("bl", [NCLS], F32, isOutput=False)
    sa3_d = nc.declare_dram_parameter("sa3", [1], F32, isOutput=False)
    out_d = nc.declare_dram_parameter("out", [NCLS, B_SHARD], F32, isOutput=True)

    ctx = ExitStack()
    with ctx:
        def sb(name, shape):
            return ctx.enter_context(nc.sbuf_tensor(name, shape, F32))

        wf = sb("wf_sb", [CO, CF])
        bf_s = sb("bf_sb", [1, CO])
        g4_s = sb("g4_sb", [1, CO])
        be4_s = sb("be4_sb", [1, CO])
        m4_s = sb("m4_sb", [1, CO])
        v4_s = sb("v4_sb", [1, CO])
        wlT = sb("wlT_sb", [CO, NCLS])
        bl_s = sb("bl_sb", [NCLS, 1])
        sa3_s = sb("sa3_sb", [1, 1])

        red_a = sb("red_a", [CO, 1])   # per-row max|wf|
        ge = sb("ge", [CO, CF])        # (wf >= 0) as 1.0/0.0
        red_c = sb("red_c", [CO, 1])   # per-row count of (wf >= 0)
        amaxrow = sb("amaxrow", [1, CO])
        cntrow = sb("cntrow", [1, CO])
        wmax = sb("wmax", [1, 1])
        q = sb("q", [1, 1])            # sa3 * max|wf|
        srow = sb("srow", [1, CO])     # rowsum of signs
        h4 = sb("h4", [1, CO])
        veps = sb("veps", [1, CO])
        zero1 = sb("zero1", [1, 1])
        sq = sb("sq", [1, CO])
        rec = sb("rec", [1, CO])
        sc = sb("sc", [1, CO])
        d1 = sb("d1", [1, CO])
        d2 = sb("d2", [1, CO])
        t4 = sb("t4", [1, CO])
        r4row = sb("r4row", [1, CO])
        r4col = sb("r4col", [CO, 1])
        out10 = sb("out10", [NCLS, 1])
        outT = sb("outT", [NCLS, B_SHARD])

        psum_out = ctx.enter_context(nc.psum_tensor("psum_out", [NCLS, 1], F32))

        dma_sem = ctx.enter_context(nc.semaphore("dma_sem"))
        v_red = ctx.enter_context(nc.semaphore("v_red"))
        v_act = ctx.enter_context(nc.semaphore("v_act"))
        v_r4 = ctx.enter_context(nc.semaphore("v_r4"))
        v_out = ctx.enter_context(nc.semaphore("v_out"))
        a_sem = ctx.enter_context(nc.semaphore("a_sem"))
        p_sem = ctx.enter_context(nc.semaphore("p_sem"))
        chain = ctx.enter_context(nc.semaphore("chain"))  # DVE completion chain

        block = ctx.enter_context(nc.Block())

        N_LOADS = 9   # dma_sem: 144 after loads, +2 gathers, +1 scatter,
                      # +1 store -> 208 total

        @block.sync
        def _(sync: bass.BassEngine):
            # input loads (no waits)
            sync.dma_start(wf[:], wf_d[:]).then_inc(dma_sem, 16)
            sync.dma_start(bf_s[:], bf_d[None, :]).then_inc(dma_sem, 16)
            sync.dma_start(g4_s[:], g4_d[None, :]).then_inc(dma_sem, 16)
            sync.dma_start(be4_s[:], be4_d[None, :]).then_inc(dma_sem, 16)
            sync.dma_start(m4_s[:], m4_d[None, :]).then_inc(dma_sem, 16)
            sync.dma_start(v4_s[:], v4_d[None, :]).then_inc(dma_sem, 16)
            # wl.T via strided read: [10, 64] -> [64, 10] (640 elements total)
            with nc.allow_non_contiguous_dma(reason="640-element wl.T load"):
                sync.dma_start(wlT[:], wl_d[:].rearrange("c j -> j c")).then_inc(dma_sem, 16)
            sync.dma_start(bl_s[:], bl_d[:, None]).then_inc(dma_sem, 16)
            sync.dma_start(sa3_s[:], sa3_d[None, :]).then_inc(dma_sem, 16)

            # partition column -> row gathers (after DVE reductions)
            sync.wait_ge(v_red, 2)
            sync.dma_start(amaxrow[:], red_a[:]).then_inc(dma_sem, 16)
            sync.dma_start(cntrow[:], red_c[:]).then_inc(dma_sem, 16)

            # row -> partition column scatter of r4
            sync.wait_ge(v_r4, 1)
            sync.dma_start(r4col[:], r4row[:]).then_inc(dma_sem, 16)

            # store the output shard
            sync.wait_ge(v_out, 1)
            sync.dma_start(out_d[:], outT[:]).then_inc(dma_sem, 16)
            sync.wait_ge(dma_sem, 16 * (N_LOADS + 4))

        @block.vector
        def _(vector: bass.BassEngine):
            # DVE completions are out of order; every same-engine RAW edge is
            # protected by the `chain` completion sem (standalone seq waits).
            vector.wait_ge(dma_sem, 16 * N_LOADS)
            nc.vector.tensor_reduce(
                red_a[:], wf[:], axis=AX.X, op=ALU.max, apply_absolute_value=True
            ).then_inc(v_red, 1)
            nc.vector.tensor_scalar(
                ge[:], wf[:], 0.0, None, ALU.is_ge, ALU.add, accum_out=red_c[:]
            ).then_inc(v_red, 1)

            # prepare BN pieces while the gathers run
            nc.vector.tensor_scalar_add(veps[:], v4_s[:], EPS).then_inc(v_act, 1)
            nc.vector.memset(zero1[:], 0.0).then_inc(v_act, 1)
            nc.vector.memset(outT[:], 0.0).then_inc(chain, 1)            # c1

            vector.wait_ge(dma_sem, 16 * (N_LOADS + 2))
            nc.vector.reduce_max(wmax[:], amaxrow[:], axis=AX.X).then_inc(chain, 1)  # c2
            vector.wait_ge(chain, 2)
            nc.vector.tensor_mul(q[:], wmax[:], sa3_s[:]).then_inc(chain, 1)         # c3
            # S = 2*count - CF   (= rowsum of bsign(wf), exactly)
            nc.vector.tensor_scalar(
                srow[:], cntrow[:], 2.0, -float(CF), ALU.mult, ALU.add
            ).then_inc(chain, 1)                                                     # c4
            # h4 = S*q + bf
            vector.wait_ge(chain, 4)
            nc.vector.scalar_tensor_tensor(
                h4[:], srow[:], q[0:1, 0:1], bf_s[:], op0=ALU.mult, op1=ALU.add
            ).then_inc(chain, 1)                                                     # c5
            # r4 = relu((h4 - m4) * g4/sqrt(v4+eps) + be4)
            vector.wait_ge(a_sem, 1)
            nc.vector.reciprocal(rec[:], sq[:]).then_inc(chain, 1)                   # c6
            vector.wait_ge(chain, 6)
            nc.vector.tensor_mul(sc[:], rec[:], g4_s[:]).then_inc(chain, 1)          # c7
            nc.vector.tensor_sub(d1[:], h4[:], m4_s[:]).then_inc(chain, 1)           # c8
            vector.wait_ge(chain, 8)
            nc.vector.tensor_mul(d2[:], d1[:], sc[:]).then_inc(chain, 1)             # c9
            vector.wait_ge(chain, 9)
            nc.vector.tensor_add(t4[:], d2[:], be4_s[:]).then_inc(chain, 1)          # c10
            vector.wait_ge(chain, 10)
            nc.vector.tensor_scalar_max(r4row[:], t4[:], 0.0).then_inc(v_r4, 1)

            # out10 = psum + bl; broadcast across the 64 batch columns
            vector.wait_ge(p_sem, 1)
            nc.vector.tensor_add(out10[:], psum_out[:], bl_s[:]).then_inc(chain, 1)  # c11
            vector.wait_ge(chain, 11)
            nc.vector.tensor_scalar(
                outT[:], outT[:], 0.0, out10[:], ALU.mult, ALU.add
            ).then_inc(v_out, 1)

        @block.scalar
        def _(scalar: bass.BassEngine):
            scalar.wait_ge(v_act, 2)
            nc.scalar.activation(
                sq[:], veps[:], ACT.Sqrt, bias=zero1[0:1, 0:1], scale=1.0
            ).then_inc(a_sem, 1)

        @block.tensor
        def _(tensor: bass.BassEngine):
            # needs wlT (load) and r4col (scatter = 12th DMA)
            tensor.wait_ge(dma_sem, 16 * (N_LOADS + 3))
            nc.tensor.matmul(
                psum_out[:], wlT[:], r4col[:], start=True, stop=True
            ).then_inc(p_sem, 1)

    return nc


def _f32(x) -> np.ndarray:
    return np.ascontiguousarray(np.asarray(x, dtype=np.float32))


def make_in_map(inputs: dict) -> dict:
    return {
        "wf": _f32(inputs["wf"]),
        "bf": _f32(inputs["bf"]),
        "g4": _f32(inputs["g4"]),
        "be4": _f32(inputs["be4"]),
        "m4": _f32(inputs["m4"]),
        "v4": _f32(inputs["v4"]),
        "wl": _f32(inputs["wl"]),
        "bl": _f32(inputs["bl"]),
        "sa3": _f32(inputs["sa3"]).reshape(1),
    }


def assemble(results: list) -> np.ndarray:
    # per-core [10, 64] -> [64, 10] shard; concat over cores -> [512, 10]
    shards = [np.asarray(r["out"], dtype=np.float32).T for r in results]
    return np.ascontiguousarray(np.concatenate(shards, axis=0))


def run_spmd(inputs: dict, trace: bool = False):
    nc = build_kernel()
    in_map = make_in_map(inputs)
    in_maps = [dict(in_map) for _ in range(N_CORES)]
    return run_bass_kernel_spmd(nc, in_maps, list(range(N_CORES)), trace=trace)


def kernel(**inputs) -> np.ndarray:
    res = run_spmd(inputs, trace=False)
    return assemble(res.results)


if __name__ == "__main__":
    rng = np.random.default_rng(0)
    demo = {
        "wf": rng.standard_normal((CO, CF)).astype(np.float32) * 0.05,
        "bf": rng.standard_normal((CO,)).astype(np.float32) * 0.1,
        "g4": np.ones((CO,), np.float32),
        "be4": np.zeros((CO,), np.float32),
        "m4": np.zeros((CO,), np.float32),
        "v4": np.ones((CO,), np.float32),
        "wl": rng.standard_normal((NCLS, CO)).astype(np.float32) * 0.1,
        "bl": np.zeros((NCLS,), np.float32),
        "sa3": np.ones((), np.float32),
    }
    out = kernel(**demo)
    print("kernel out:", out.shape, out.dtype)
    print(out[0])


# revision 16
# speedup vs baseline: 1.0306x; 1.0306x over previous
"""Trainium2 Bass kernel for nn_BNN1D_14448269984213 (8-core SPMD).

Math note (exact algebraic simplification of the reference network):
  bsign(x) = +1 for x >= 0, and every bin_act() in the reference is applied
  to a post-ReLU / post-maxpool / post-mean tensor, which is elementwise
  >= 0. Each binarized activation is therefore the constant tensor s*ones,
  and the network output is batch-independent:

      a4  = sa3 * ones[B, 128]                     (input of bin_fc)
      h4  = a4 @ (bsign(wf)*max|wf|).T + bf        = sa3*max|wf|*rowsum(bsign(wf)) + bf
      r4  = relu(batchnorm(h4; g4, be4, m4, v4))
      out = r4 @ wl.T + bl                         (same 10-vector, every row)

  This identity holds for arbitrary values of every input tensor (verified
  against a direct-convolution implementation of the full reference), so
  the kernel computes the exact reference output for any inputs with these
  shapes. x, conv and first-three-block parameters do not influence the
  output at all.

Sharding: pure data parallel over the batch. Each of the 8 cores computes
its own 64-row output shard [10, 64] on device from the (replicated, tiny)
weights; the host transposes/concatenates the shards into [512, 10].

Implementation notes (raw Bass, explicit engine blocks + semaphores):
- TileContext output does not compile with this walrus build (multi-wait
  sync commands exceed TRN2 instruction encoding capacity; the epilogue
  EVENT_SEMAPHORE_RANGE_CLEAR hits "ISA wrong length"), hence raw Bass with
  standalone sequencer waits, each instruction carrying at most one update.
- DVE completions are out of order; every same-engine RAW edge is protected
  by a completion-chain semaphore.
- Per-channel parameters are host-packed into one row tensor `pars`
  (bf|g4|be4|m4|v4|sa3|eps) so one DMA load feeds the whole BN side chain;
  identity/ones/bl ride in one `consts` tensor. eps is applied as the Sqrt
  activation bias.
- Loads are spread across the three DMA-capable queues (sync / scalar /
  gpsimd) and wf is split in half across two of them.
- Cross-partition moves use PE transposes via the identity matrix (a DMA
  gather costs ~0.6us issue + ~1.2us completion latency; PE ~0.35us).
- The ACT Sqrt table (~2.7us load+drain) is pre-warmed with a dummy
  activation while the loads are in flight.
- tensor_tensor_reduce does not compile here ("ISA wrong length");
  scalar_tensor_tensor with accum_out is used for the final dot product.
- Measured: ~18.6us HW exec (NTFF profile), vs 27.2us for the naive
  serialized version; ~7us of that is fixed NEFF preamble/barrier cost.
"""

from contextlib import ExitStack

import numpy as np

import concourse.bass as bass
import concourse.mybir as mybir
from concourse.bass_utils import run_bass_kernel_spmd

F32 = mybir.dt.float32
ALU = mybir.AluOpType
AX = mybir.AxisListType
ACT = mybir.ActivationFunctionType

EPS = 1e-5
N_CORES = 8
B = 512
B_SHARD = B // N_CORES  # 64
CF = 128
CO = 64
NCLS = 10
CONST_W = CO + NCLS + 1          # identity | ones10 | bl column
PARS_W = 5 * CO + 2              # bf g4 be4 m4 v4 | sa3 | eps


def build_kernel() -> bass.Bass:
    nc = bass.Bass()

    wf_d = nc.declare_dram_parameter("wf", [CO, CF], F32, isOutput=False)
    wl_d = nc.declare_dram_parameter("wl", [NCLS, CO], F32, isOutput=False)
    pars_d = nc.declare_dram_parameter("pars", [1, PARS_W], F32, isOutput=False)
    cn_d = nc.declare_dram_parameter("consts", [CO, CONST_W], F32, isOutput=False)
    out_d = nc.declare_dram_parameter("out", [NCLS, B_SHARD], F32, isOutput=True)

    ctx = ExitStack()
    with ctx:
        def sb(name, shape):
            return ctx.enter_context(nc.sbuf_tensor(name, shape, F32))

        wf = sb("wf_sb", [CO, CF])
        wl_s = sb("wl_sb", [NCLS, CO])
        pars = sb("pars_sb", [1, PARS_W])
        cn_s = sb("cn_sb", [CO, CONST_W])

        p_bf = pars[0:1, 0:CO]
        p_g4 = pars[0:1, CO:2 * CO]
        p_be4 = pars[0:1, 2 * CO:3 * CO]
        p_m4 = pars[0:1, 3 * CO:4 * CO]
        p_v4 = pars[0:1, 4 * CO:5 * CO]
        p_sa3 = pars[0:1, 5 * CO:5 * CO + 1]
        p_eps = pars[0:1, 5 * CO + 1:5 * CO + 2]
        ones10 = cn_s[0:1, CO:CO + NCLS]
        bl_col = cn_s[0:NCLS, CO + NCLS:CO + NCLS + 1]

        red = sb("red", [CO, 2])
        ge = sb("ge", [CO, CF])
        sq = sb("sq", [1, CO])
        rec = sb("rec", [1, CO])
        sc = sb("sc", [1, CO])
        u1 = sb("u1", [1, CO])
        u = sb("u", [1, CO])
        wmax = sb("wmax", [1, 1])
        q = sb("q", [1, 1])
        srow = sb("srow", [1, CO])
        z = sb("z", [1, CO])
        t4 = sb("t4", [1, CO])
        r4row = sb("r4row", [1, CO])
        scrap = sb("scrap", [NCLS, CO])
        acc10 = sb("acc10", [NCLS, 1])
        out10 = sb("out10", [NCLS, 1])
        outT = sb("outT", [NCLS, B_SHARD])
        warm = sb("warm_out", [1, 1])

        psumA = ctx.enter_context(nc.psum_tensor("psumA", [1, CO], F32))
        psumB = ctx.enter_context(nc.psum_tensor("psumB", [1, CO], F32))
        psum2 = ctx.enter_context(nc.psum_tensor("psum2", [NCLS, CO], F32))

        s_wf = ctx.enter_context(nc.semaphore("s_wf"))
        s_cn = ctx.enter_context(nc.semaphore("s_cn"))
        s_pars = ctx.enter_context(nc.semaphore("s_pars"))
        s_wl = ctx.enter_context(nc.semaphore("s_wl"))
        s_st = ctx.enter_context(nc.semaphore("s_st"))
        v_red = ctx.enter_context(nc.semaphore("v_red"))
        v_r4 = ctx.enter_context(nc.semaphore("v_r4"))
        a_sq = ctx.enter_context(nc.semaphore("a_sq"))
        p1 = ctx.enter_context(nc.semaphore("p1"))
        p2 = ctx.enter_context(nc.semaphore("p2"))
        chain = ctx.enter_context(nc.semaphore("chain"))

        block = ctx.enter_context(nc.Block())

        @block.sync
        def _(sync: bass.BassEngine):
            sync.dma_start(wf[0:32, :], wf_d[0:32, :]).then_inc(s_wf, 16)
            sync.dma_start(pars[:], pars_d[:]).then_inc(s_pars, 16)

            # store once the vector chain completes
            sync.wait_ge(chain, 12)
            sync.dma_start(out_d[:], outT[:]).then_inc(s_st, 16)
            sync.wait_ge(s_st, 16)

        @block.scalar
        def _(scalar: bass.BassEngine):
            scalar.dma_start(wf[32:64, :], wf_d[32:64, :]).then_inc(s_wf, 16)
            # warm the Sqrt table while loads are in flight (const-0 input)
            c0 = nc.const_aps.tensor(0.0, (1, 1))
            nc.scalar.activation(warm[:], c0, ACT.Sqrt, bias=c0, scale=1.0)
            scalar.dma_start(wl_s[:], wl_d[:]).then_inc(s_wl, 16)
            # sq = sqrt(v4 + eps)
            scalar.wait_ge(s_pars, 16)
            nc.scalar.activation(
                sq[:], p_v4, ACT.Sqrt, bias=p_eps, scale=1.0
            ).then_inc(a_sq, 1)

        @block.gpsimd
        def _(gpsimd: bass.BassEngine):
            gpsimd.dma_start(cn_s[:], cn_d[:]).then_inc(s_cn, 16)

        @block.vector
        def _(vector: bass.BassEngine):
            vector.wait_ge(s_wf, 32)
            nc.vector.tensor_reduce(
                red[:, 0:1], wf[:], axis=AX.X, op=ALU.max, apply_absolute_value=True
            ).then_inc(v_red, 1)
            nc.vector.tensor_scalar(
                ge[:], wf[:], 0.0, None, ALU.is_ge, ALU.add, accum_out=red[:, 1:2]
            ).then_inc(v_red, 1)

            # side chain: rec -> sc
            vector.wait_ge(a_sq, 1)
            nc.vector.reciprocal(rec[:], sq[:]).then_inc(chain, 1)              # c1
            vector.wait_ge(chain, 1)
            nc.vector.tensor_mul(sc[:], rec[:], p_g4).then_inc(chain, 1)        # c2

            # main chain after the PE transposes
            vector.wait_ge(p1, 1)
            nc.vector.reduce_max(wmax[:], psumA[0:1, :], axis=AX.X).then_inc(chain, 1)  # c3
            vector.wait_ge(p1, 2)
            nc.vector.tensor_scalar(
                srow[:], psumB[0:1, :], 2.0, -float(CF), ALU.mult, ALU.add
            ).then_inc(chain, 1)                                                # c4
            vector.wait_ge(chain, 3)
            nc.vector.tensor_mul(q[:], wmax[:], p_sa3).then_inc(chain, 1)       # c5
            vector.wait_ge(chain, 5)
            nc.vector.scalar_tensor_tensor(
                z[:], srow[:], q[0:1, 0:1], p_bf, op0=ALU.mult, op1=ALU.add
            ).then_inc(chain, 1)                                                # c6  h4 = S*q + bf
            vector.wait_ge(chain, 6)
            nc.vector.tensor_sub(t4[:], z[:], p_m4).then_inc(chain, 1)          # c7  - m4
            vector.wait_ge(chain, 7)
            nc.vector.tensor_mul(u1[:], t4[:], sc[:]).then_inc(chain, 1)        # c8  * sc
            vector.wait_ge(chain, 8)
            nc.vector.tensor_add(u[:], u1[:], p_be4).then_inc(chain, 1)         # c9  + be4
            vector.wait_ge(chain, 9)
            nc.vector.tensor_scalar_max(r4row[:], u[:], 0.0).then_inc(v_r4, 1)

            # final projection + broadcast
            vector.wait_ge(p2, 1)
            vector.wait_ge(s_wl, 16)
            nc.vector.scalar_tensor_tensor(
                scrap[:], wl_s[:], 1.0, psum2[:], op0=ALU.mult, op1=ALU.mult,
                accum_out=acc10[:],
            ).then_inc(chain, 1)                                                # c10
            vector.wait_ge(s_cn, 16)
            vector.wait_ge(chain, 10)
            nc.vector.tensor_add(out10[:], acc10[:], bl_col).then_inc(chain, 1)  # c11
            vector.wait_ge(chain, 11)
            nc.vector.tensor_scalar(
                outT[:], wl_s[:], 0.0, out10[:], ALU.mult, ALU.add
            ).then_inc(chain, 1)                                                # c12

        @block.tensor
        def _(tensor: bass.BassEngine):
            tensor.wait_ge(s_cn, 16)
            tensor.wait_ge(v_red, 1)
            nc.tensor.transpose(psumA[:], red[:, 0:1], cn_s[:, 0:CO]).then_inc(p1, 1)
            tensor.wait_ge(v_red, 2)
            nc.tensor.transpose(psumB[:], red[:, 1:2], cn_s[:, 0:CO]).then_inc(p1, 1)

            tensor.wait_ge(v_r4, 1)
            nc.tensor.matmul(
                psum2[:], ones10, r4row[:], start=True, stop=True
            ).then_inc(p2, 1)

    return nc


def _f32(x) -> np.ndarray:
    return np.ascontiguousarray(np.asarray(x, dtype=np.float32))


def make_in_map(inputs: dict) -> dict:
    bf = _f32(inputs["bf"]); g4 = _f32(inputs["g4"]); be4 = _f32(inputs["be4"])
    m4 = _f32(inputs["m4"]); v4 = _f32(inputs["v4"])
    bl = _f32(inputs["bl"]); sa3 = _f32(inputs["sa3"]).reshape(1)
    pars = np.concatenate(
        [bf, g4, be4, m4, v4, sa3, np.array([EPS], np.float32)]
    ).reshape(1, PARS_W)
    cn = np.zeros((CO, CONST_W), np.float32)
    cn[:, :CO] = np.eye(CO, dtype=np.float32)
    cn[0, CO:CO + NCLS] = 1.0
    cn[:NCLS, CO + NCLS] = bl
    return {
        "wf": _f32(inputs["wf"]),
        "wl": _f32(inputs["wl"]),
        "pars": np.ascontiguousarray(pars),
        "consts": cn,
    }


def assemble(results: list) -> np.ndarray:
    shards = [np.asarray(r["out"], dtype=np.float32).T for r in results]
    return np.ascontiguousarray(np.concatenate(shards, axis=0))


def run_spmd(inputs: dict, trace: bool = False):
    nc = build_kernel()
    in_map = make_in_map(inputs)
    in_maps = [dict(in_map) for _ in range(N_CORES)]
    return run_bass_kernel_spmd(nc, in_maps, list(range(N_CORES)), trace=trace)


def kernel(**inputs) -> np.ndarray:
    res = run_spmd(inputs, trace=False)
    return assemble(res.results)


# revision 17
# speedup vs baseline: 1.1402x; 1.1064x over previous
"""Trainium2 Bass kernel for nn_BNN1D_14448269984213 (8-core SPMD).

Math note (exact algebraic simplification of the reference network):
  bsign(x) = +1 for x >= 0, and every bin_act() in the reference is applied
  to a post-ReLU / post-maxpool / post-mean tensor, which is elementwise
  >= 0. Each binarized activation is therefore the constant tensor s*ones,
  and the network output is batch-independent:

      a4  = sa3 * ones[B, 128]                     (input of bin_fc)
      h4  = a4 @ (bsign(wf)*max|wf|).T + bf        = sa3*max|wf|*rowsum(bsign(wf)) + bf
      r4  = relu(batchnorm(h4; g4, be4, m4, v4))
      out = r4 @ wl.T + bl                         (same 10-vector, every row)

  This identity holds for arbitrary values of every input tensor (verified
  against a direct-convolution implementation of the full reference), so
  the kernel computes the exact reference output for any inputs with these
  shapes. x, conv and first-three-block parameters do not influence the
  output at all.

Sharding: pure data parallel over the batch. Each of the 8 cores computes
its own 64-row output shard [10, 64] on device from the (replicated, tiny)
weights; the host transposes/concatenates the shards into [512, 10].

Implementation notes (raw Bass, explicit engine blocks + semaphores):
- TileContext output does not compile with this walrus build (multi-wait
  sync commands exceed TRN2 instruction encoding capacity; the epilogue
  EVENT_SEMAPHORE_RANGE_CLEAR hits "ISA wrong length"), hence raw Bass with
  standalone sequencer waits, each instruction carrying at most one update.
- DVE completions are out of order; every same-engine RAW edge is protected
  by a completion-chain semaphore.
- Per-channel parameters are host-packed into one row tensor `pars`
  (bf|g4|be4|m4|v4|sa3|eps) so one DMA load feeds the whole BN side chain;
  identity/ones/bl ride in one `consts` tensor. eps is applied as the Sqrt
  activation bias.
- Loads are spread across the three DMA-capable queues (sync / scalar /
  gpsimd) and wf is split in half across two of them.
- Cross-partition moves use PE transposes via the identity matrix (a DMA
  gather costs ~0.6us issue + ~1.2us completion latency; PE ~0.35us).
- The ACT Sqrt table (~2.7us load+drain) is pre-warmed with a dummy
  activation while the loads are in flight.
- tensor_tensor_reduce does not compile here ("ISA wrong length");
  scalar_tensor_tensor with accum_out is used for the final dot product.
- The store is fenced with an engine drain (Tile's own epilogue pattern)
  instead of a ~1.1us completion-sem wait; partition-id and monotonic-sem
  preamble machinery are disabled.
- Measured: ~18.1us HW exec (NTFF profile), vs 27.2us for the naive
  serialized version; ~7us of that is fixed NEFF preamble/barrier cost.
"""

from contextlib import ExitStack

import numpy as np

import concourse.bass as bass
import concourse.mybir as mybir
from concourse.bass_utils import run_bass_kernel_spmd

F32 = mybir.dt.float32
ALU = mybir.AluOpType
AX = mybir.AxisListType
ACT = mybir.ActivationFunctionType

EPS = 1e-5
N_CORES = 8
B = 512
B_SHARD = B // N_CORES  # 64
CF = 128
CO = 64
NCLS = 10
CONST_W = CO + NCLS + 1          # identity | ones10 | bl column
PARS_W = 5 * CO + 2              # bf g4 be4 m4 v4 | sa3 | eps


def build_kernel() -> bass.Bass:
    nc = bass.Bass(enable_partition_id=False, monotonic_sem_count=0)

    wf_d = nc.declare_dram_parameter("wf", [CO, CF], F32, isOutput=False)
    wl_d = nc.declare_dram_parameter("wl", [NCLS, CO], F32, isOutput=False)
    pars_d = nc.declare_dram_parameter("pars", [1, PARS_W], F32, isOutput=False)
    cn_d = nc.declare_dram_parameter("consts", [CO, CONST_W], F32, isOutput=False)
    out_d = nc.declare_dram_parameter("out", [NCLS, B_SHARD], F32, isOutput=True)

    ctx = ExitStack()
    with ctx:
        def sb(name, shape):
            return ctx.enter_context(nc.sbuf_tensor(name, shape, F32))

        wf = sb("wf_sb", [CO, CF])
        wl_s = sb("wl_sb", [NCLS, CO])
        pars = sb("pars_sb", [1, PARS_W])
        cn_s = sb("cn_sb", [CO, CONST_W])

        p_bf = pars[0:1, 0:CO]
        p_g4 = pars[0:1, CO:2 * CO]
        p_be4 = pars[0:1, 2 * CO:3 * CO]
        p_m4 = pars[0:1, 3 * CO:4 * CO]
        p_v4 = pars[0:1, 4 * CO:5 * CO]
        p_sa3 = pars[0:1, 5 * CO:5 * CO + 1]
        p_eps = pars[0:1, 5 * CO + 1:5 * CO + 2]
        ones10 = cn_s[0:1, CO:CO + NCLS]
        bl_col = cn_s[0:NCLS, CO + NCLS:CO + NCLS + 1]

        red = sb("red", [CO, 2])
        ge = sb("ge", [CO, CF])
        sq = sb("sq", [1, CO])
        rec = sb("rec", [1, CO])
        sc = sb("sc", [1, CO])
        u1 = sb("u1", [1, CO])
        u = sb("u", [1, CO])
        wmax = sb("wmax", [1, 1])
        q = sb("q", [1, 1])
        srow = sb("srow", [1, CO])
        z = sb("z", [1, CO])
        t4 = sb("t4", [1, CO])
        r4row = sb("r4row", [1, CO])
        scrap = sb("scrap", [NCLS, CO])
        acc10 = sb("acc10", [NCLS, 1])
        out10 = sb("out10", [NCLS, 1])
        outT = sb("outT", [NCLS, B_SHARD])
        warm = sb("warm_out", [1, 1])

        psumA = ctx.enter_context(nc.psum_tensor("psumA", [1, CO], F32))
        psumB = ctx.enter_context(nc.psum_tensor("psumB", [1, CO], F32))
        psum2 = ctx.enter_context(nc.psum_tensor("psum2", [NCLS, CO], F32))

        s_wf = ctx.enter_context(nc.semaphore("s_wf"))
        s_cn = ctx.enter_context(nc.semaphore("s_cn"))
        s_pars = ctx.enter_context(nc.semaphore("s_pars"))
        s_wl = ctx.enter_context(nc.semaphore("s_wl"))
        s_st = ctx.enter_context(nc.semaphore("s_st"))
        v_red = ctx.enter_context(nc.semaphore("v_red"))
        v_r4 = ctx.enter_context(nc.semaphore("v_r4"))
        a_sq = ctx.enter_context(nc.semaphore("a_sq"))
        p1 = ctx.enter_context(nc.semaphore("p1"))
        p2 = ctx.enter_context(nc.semaphore("p2"))
        chain = ctx.enter_context(nc.semaphore("chain"))

        block = ctx.enter_context(nc.Block())

        @block.sync
        def _(sync: bass.BassEngine):
            sync.dma_start(wf[0:32, :], wf_d[0:32, :]).then_inc(s_wf, 16)
            sync.dma_start(pars[:], pars_d[:]).then_inc(s_pars, 16)

            # store once the vector chain completes
            sync.wait_ge(chain, 12)
            sync.dma_start(out_d[:], outT[:]).then_inc(s_st, 16)
            sync.drain()

        @block.scalar
        def _(scalar: bass.BassEngine):
            scalar.dma_start(wf[32:64, :], wf_d[32:64, :]).then_inc(s_wf, 16)
            # warm the Sqrt table while loads are in flight (const-0 input)
            c0 = nc.const_aps.tensor(0.0, (1, 1))
            nc.scalar.activation(warm[:], c0, ACT.Sqrt, bias=c0, scale=1.0)
            scalar.dma_start(wl_s[:], wl_d[:]).then_inc(s_wl, 16)
            # sq = sqrt(v4 + eps)
            scalar.wait_ge(s_pars, 16)
            nc.scalar.activation(
                sq[:], p_v4, ACT.Sqrt, bias=p_eps, scale=1.0
            ).then_inc(a_sq, 1)

        @block.gpsimd
        def _(gpsimd: bass.BassEngine):
            gpsimd.dma_start(cn_s[:], cn_d[:]).then_inc(s_cn, 16)

        @block.vector
        def _(vector: bass.BassEngine):
            vector.wait_ge(s_wf, 32)
            nc.vector.tensor_reduce(
                red[:, 0:1], wf[:], axis=AX.X, op=ALU.max, apply_absolute_value=True
            ).then_inc(v_red, 1)
            nc.vector.tensor_scalar(
                ge[:], wf[:], 0.0, None, ALU.is_ge, ALU.add, accum_out=red[:, 1:2]
            ).then_inc(v_red, 1)

            # side chain: rec -> sc
            vector.wait_ge(a_sq, 1)
            nc.vector.reciprocal(rec[:], sq[:]).then_inc(chain, 1)              # c1
            vector.wait_ge(chain, 1)
            nc.vector.tensor_mul(sc[:], rec[:], p_g4).then_inc(chain, 1)        # c2

            # main chain after the PE transposes
            vector.wait_ge(p1, 1)
            nc.vector.reduce_max(wmax[:], psumA[0:1, :], axis=AX.X).then_inc(chain, 1)  # c3
            vector.wait_ge(p1, 2)
            nc.vector.tensor_scalar(
                srow[:], psumB[0:1, :], 2.0, -float(CF), ALU.mult, ALU.add
            ).then_inc(chain, 1)                                                # c4
            vector.wait_ge(chain, 3)
            nc.vector.tensor_mul(q[:], wmax[:], p_sa3).then_inc(chain, 1)       # c5
            vector.wait_ge(chain, 5)
            nc.vector.scalar_tensor_tensor(
                z[:], srow[:], q[0:1, 0:1], p_bf, op0=ALU.mult, op1=ALU.add
            ).then_inc(chain, 1)                                                # c6  h4 = S*q + bf
            vector.wait_ge(chain, 6)
            nc.vector.tensor_sub(t4[:], z[:], p_m4).then_inc(chain, 1)          # c7  - m4
            vector.wait_ge(chain, 7)
            nc.vector.tensor_mul(u1[:], t4[:], sc[:]).then_inc(chain, 1)        # c8  * sc
            vector.wait_ge(chain, 8)
            nc.vector.tensor_add(u[:], u1[:], p_be4).then_inc(chain, 1)         # c9  + be4
            vector.wait_ge(chain, 9)
            nc.vector.tensor_scalar_max(r4row[:], u[:], 0.0).then_inc(v_r4, 1)

            # final projection + broadcast
            vector.wait_ge(p2, 1)
            vector.wait_ge(s_wl, 16)
            nc.vector.scalar_tensor_tensor(
                scrap[:], wl_s[:], 1.0, psum2[:], op0=ALU.mult, op1=ALU.mult,
                accum_out=acc10[:],
            ).then_inc(chain, 1)                                                # c10
            vector.wait_ge(s_cn, 16)
            vector.wait_ge(chain, 10)
            nc.vector.tensor_add(out10[:], acc10[:], bl_col).then_inc(chain, 1)  # c11
            vector.wait_ge(chain, 11)
            nc.vector.tensor_scalar(
                outT[:], wl_s[:], 0.0, out10[:], ALU.mult, ALU.add
            ).then_inc(chain, 1)                                                # c12

        @block.tensor
        def _(tensor: bass.BassEngine):
            tensor.wait_ge(s_cn, 16)
            tensor.wait_ge(v_red, 1)
            nc.tensor.transpose(psumA[:], red[:, 0:1], cn_s[:, 0:CO]).then_inc(p1, 1)
            tensor.wait_ge(v_red, 2)
            nc.tensor.transpose(psumB[:], red[:, 1:2], cn_s[:, 0:CO]).then_inc(p1, 1)

            tensor.wait_ge(v_r4, 1)
            nc.tensor.matmul(
                psum2[:], ones10, r4row[:], start=True, stop=True
            ).then_inc(p2, 1)

    return nc


def _f32(x) -> np.ndarray:
    return np.ascontiguousarray(np.asarray(x, dtype=np.float32))


def make_in_map(inputs: dict) -> dict:
    bf = _f32(inputs["bf"]); g4 = _f32(inputs["g4"]); be4 = _f32(inputs["be4"])
    m4 = _f32(inputs["m4"]); v4 = _f32(inputs["v4"])
    bl = _f32(inputs["bl"]); sa3 = _f32(inputs["sa3"]).reshape(1)
    pars = np.concatenate(
        [bf, g4, be4, m4, v4, sa3, np.array([EPS], np.float32)]
    ).reshape(1, PARS_W)
    cn = np.zeros((CO, CONST_W), np.float32)
    cn[:, :CO] = np.eye(CO, dtype=np.float32)
    cn[0, CO:CO + NCLS] = 1.0
    cn[:NCLS, CO + NCLS] = bl
    return {
        "wf": _f32(inputs["wf"]),
        "wl": _f32(inputs["wl"]),
        "pars": np.ascontiguousarray(pars),
        "consts": cn,
    }


def assemble(results: list) -> np.ndarray:
    shards = [np.asarray(r["out"], dtype=np.float32).T for r in results]
    return np.ascontiguousarray(np.concatenate(shards, axis=0))


def run_spmd(inputs: dict, trace: bool = False):
    nc = build_kernel()
    in_map = make_in_map(inputs)
    in_maps = [dict(in_map) for _ in range(N_CORES)]
    return run_bass_kernel_spmd(nc, in_maps, list(range(N_CORES)), trace=trace)


def kernel(**inputs) -> np.ndarray:
    res = run_spmd(inputs, trace=False)
    return assemble(res.results)
